# revision 22
# baseline (speedup 1.0000x reference)
"""Trainium2 Bass kernel for a causal attention block with softmax over the
QUERY axis (axis=1), data-parallel over batch across 8 NeuronCores.

Reference semantics (per batch element):
    q = x @ Wq + bq ; k = x @ Wk + bk ; v = x @ Wv + bv        # [T, 512]
    logits[t, s] = q[t] . k[s]   masked to s <= t (causal)
    probs = softmax(logits / sqrt(512), axis=t)                # query axis!
    read[t] = sum_s probs[t, s] * v[s]
    out = concat(x, read)

Device kernel computes `read` for one batch element; the batch is sharded
1-per-core across 8 cores and the x-passthrough concat happens on host.

Key layout choice: we materialize the score matrix TRANSPOSED,
L'[s, t] = q[t].k[s], so the softmax reduction (over t) runs along the free
axis, and L' (post-exp) feeds the read matmul directly as the stationary
operand: read[t, v] = sum_s P'[s, t] * vn[s, v] with vn = v / r (the softmax
denominator r[s] is folded into v instead of normalizing the big matrix).
"""

import math
from contextlib import ExitStack

import numpy as np

T = 2048
C = 512  # input channels (contract dim of projections)
K = 512  # key dim (contract dim of logits)
V = 512
P = 128
NCORES = 8
SCALE = 1.0 / math.sqrt(float(K))
NEG = -1.0e9

NT = T // P  # 16 t-chunks (and s-chunks)
NCC = C // P  # 4 contract chunks for projections
NKC = K // P  # 4 contract chunks for logits
NJ = T // 512  # 4 512-wide t slices

# P' strip i covers t in [T0[i], T); strips stored back-to-back (bf16)
T0 = [512 * (i // 4) for i in range(NT)]
WIDTHS = [T - T0[i] for i in range(NT)]
OFFS = np.cumsum([0] + WIDTHS).tolist()  # OFFS[16] == 20480


def emit(tc, out_ap, x, wq, bq, wk, bk, wv, bv):
    import concourse.bass as bass
    import concourse.mybir as mybir
    from concourse.masks import make_identity

    nc = tc.nc
    f32 = mybir.dt.float32
    f32r = mybir.dt.float32r
    bf16 = mybir.dt.bfloat16
    Exp = mybir.ActivationFunctionType.Exp
    Ident = mybir.ActivationFunctionType.Identity
    AX = mybir.AxisListType.X
    ADD = mybir.AluOpType.add

    with ExitStack() as ctx:
        const = ctx.enter_context(tc.tile_pool(name="const", bufs=1))
        # one 4-slot pool of 8KB/partition tiles, time-shared in lifetime
        # order: xg0-3 -> wq, wk -> WqT, WkT -> W2 -> wv
        pa = ctx.enter_context(tc.tile_pool(name="pa", bufs=4))
        xtp = ctx.enter_context(tc.tile_pool(name="xtp", bufs=1))
        gtp = ctx.enter_context(tc.tile_pool(name="gtp", bufs=1))
        ppp = ctx.enter_context(tc.tile_pool(name="ppp", bufs=1))
        vpool = ctx.enter_context(tc.tile_pool(name="vpool", bufs=1))
        stat = ctx.enter_context(tc.tile_pool(name="stat", bufs=4))
        outp = ctx.enter_context(tc.tile_pool(name="outp", bufs=2))
        psum = ctx.enter_context(tc.tile_pool(name="psum", bufs=8, space="PSUM"))

        # ---- input DMAs on the sync HWDGE queue, ordered by need-time ----
        def load_xg(tg):
            xg = pa.tile([P, 4, C], f32r, tag="pa", name=f"xg{tg}")
            nc.sync.dma_start(
                xg, x[512 * tg : 512 * (tg + 1), :].rearrange("(a p) c -> p a c", p=P)
            )
            return xg

        xgs = [load_xg(0)]

        # ---- PE warm-up: dummy bf16 matmuls defeat the HAM cold-start ----
        dummy = const.tile([P, 512], bf16)
        nc.vector.memset(dummy, 0.0)

        def warm_mm():
            # rotating psum tile: don't pin one bank for the whole early phase
            dps = psum.tile([P, 512], f32, tag="ps", name="dps")
            nc.tensor.matmul(dps, dummy[:, 0:P], dummy, start=True, stop=True)

        for _ in range(6):
            warm_mm()

        # wv has a permanent home so its (early-needed) DMA is not slot-gated
        wv_sb = const.tile([P, NCC, 512], f32r)
        for c in range(NCC):
            nc.sync.dma_start(wv_sb[:, c, :], wv[P * c : P * (c + 1), :])
        xgs.append(load_xg(1))
        bv_bc = const.tile([P, V], f32)
        nc.sync.dma_start(
            bv_bc, bass.AP(tensor=bv.tensor, offset=bv.offset, ap=[[0, P], [1, V]])
        )
        bk_bc = const.tile([P, K], f32)
        nc.sync.dma_start(
            bk_bc, bass.AP(tensor=bk.tensor, offset=bk.offset, ap=[[0, P], [1, K]])
        )
        xgs += [load_xg(2), load_xg(3)]

        # weights: f32r in DRAM (bit-identical to f32), per-chunk DMAs.
        # wq/wk land in recycled xg slots (their DMAs wait on the slot WAR);
        # they are only needed by the W2 stage after all four x groups.
        wq_sb = pa.tile([P, NCC, 512], f32r, tag="pa")
        wk_sb = pa.tile([P, NCC, 512], f32r, tag="pa")
        for c in range(NCC):
            nc.sync.dma_start(wq_sb[:, c, :], wq[P * c : P * (c + 1), :])
        for c in range(NCC):
            nc.sync.dma_start(wk_sb[:, c, :], wk[P * c : P * (c + 1), :])

        # ---- constants ----
        identg = const.tile([P, P], f32)
        make_identity(nc, identg)
        ident = const.tile([P, P], f32r)
        nc.vector.tensor_copy(ident, identg)

        # additive causal masks for the diagonal 128x512 tiles; pattern m
        # (m = i mod 4): keep (0.0) where f >= p + 128*m else NEG
        masks = const.tile([P, 4 * 512], f32)
        nc.gpsimd.memset(masks, 0.0)
        for m in range(4):
            sl = masks[:, 512 * m : 512 * (m + 1)]
            nc.gpsimd.affine_select(
                out=sl,
                in_=sl,
                compare_op=mybir.AluOpType.is_ge,
                fill=NEG,
                base=-128 * m,
                channel_multiplier=-1,
                pattern=[[1, 512]],
            )

        def blk_transpose(dst, src_sb, jj):
            """dst[:, jj, :] = 128x512 strip of src^T (4 packed PE transposes)."""
            pt = psum.tile([P, 512], f32r, tag="ps", name="pt")
            for cc in range(4):
                nc.tensor.matmul(
                    pt[:, P * cc : P * (cc + 1)],
                    src_sb[:, cc, P * jj : P * (jj + 1)],
                    ident,
                    is_transpose=True,
                    start=(cc == 0),
                    stop=(cc == 3),
                )
            nc.vector.tensor_copy(dst[:, jj, :], pt)

        # ---- x transposed to xT [c, t] (c on partitions); the bias-product
        # trick below turns the q/k projections into ONE projection, so x is
        # both the moving operand of the scores matmul and the transpose src
        xT = xtp.tile([P, NCC * T], f32r)  # strip c at [:, T*c : T*(c+1)]
        vsb = vpool.tile([P, NT * 512], bf16)  # s-chunk i at [:, 512i:+512]
        for tg in range(4):  # groups of 4 t-chunks
            xg = xgs[tg]
            for c in range(NCC):
                pt = psum.tile([P, 512], f32r, tag="ps")
                for tb in range(4):
                    nc.tensor.matmul(
                        pt[:, P * tb : P * (tb + 1)],
                        xg[:, tb, P * c : P * (c + 1)],
                        ident,
                        is_transpose=True,
                        start=(tb == 0),
                        stop=(tb == 3),
                    )
                nc.scalar.copy(xT[:, T * c + 512 * tg : T * c + 512 * (tg + 1)], pt)
            # transposes don't count as PE-busy for the HAM clock gate
            warm_mm()
            # v projection for this group's s-chunks: real PE work that hides
            # the in-flight loads of the later x groups and wq/wk
            for i in range(4 * tg, 4 * tg + 4):
                pv = psum.tile([P, 512], f32, tag="ps")
                for c in range(NCC):
                    nc.tensor.matmul(
                        pv,
                        xT[:, T * c + P * i : T * c + P * (i + 1)],
                        wv_sb[:, c, :],
                        start=(c == 0),
                        stop=(c == NCC - 1),
                    )
                nc.vector.tensor_tensor(vsb[:, 512 * i : 512 * (i + 1)], pv, bv_bc, ADD)

        # Wq^T / Wk^T (W2 = Wk @ Wq^T needs the k-index of both operands on
        # partitions)
        wqT = pa.tile([P, NKC, 512], f32r, tag="pa", name="wqT")
        for jj in range(NKC):
            blk_transpose(wqT, wq_sb, jj)
        warm_mm()
        wkT = pa.tile([P, NKC, 512], f32r, tag="pa", name="wkT")
        for jj in range(NKC):
            blk_transpose(wkT, wk_sb, jj)
        warm_mm()

        # g0[c'] = sum_j Wq[c', j] * bk[j] (the bk half of the score bias;
        # the bq half is constant along the softmax axis and cancels exactly)
        g0t = stat.tile([P, 4], f32, tag="g0", bufs=1)
        for cc in range(NCC):
            scr = outp.tile([P, 512], f32, tag="ot", name="g0scr")
            nc.vector.tensor_tensor(
                scr, wq_sb[:, cc, :].bitcast(f32), bk_bc, mybir.AluOpType.mult
            )
            nc.vector.reduce_sum(g0t[:, cc : cc + 1], scr, axis=AX)

        # ---- W2 = Wk @ Wq^T  [c, c'] (contract over j on partitions) ----
        w2_sb = pa.tile([P, NCC, 512], f32r, tag="pa", name="w2")
        for a in range(NCC):
            pw = psum.tile([P, 512], f32, tag="ps")
            for jj in range(NKC):
                nc.tensor.matmul(
                    pw,
                    wkT[:, jj, P * a : P * (a + 1)],
                    wqT[:, jj, :],
                    start=(jj == 0),
                    stop=(jj == NKC - 1),
                )
            nc.scalar.copy(w2_sb[:, a, :], pw)

        # ---- G2^T[c', s] = sum_c W2[c, c'] xT[c, s] + g0[c'] ----
        # scores then need NO q/k projections: L'[s,t] = sum_c' G2T[c',s] xT[c',t]
        GT = gtp.tile([P, NCC * T], f32r)  # c'-chunk dd at [:, T*dd:+T]
        for dd in range(NCC):
            pjs = [psum.tile([P, 512], f32, tag="ps", name=f"pg{ss}") for ss in range(NJ)]
            for c in range(NCC):
                lhsT = w2_sb[:, c, P * dd : P * (dd + 1)]
                for ss in range(NJ):
                    nc.tensor.matmul(
                        pjs[ss],
                        lhsT,
                        xT[:, T * c + 512 * ss : T * c + 512 * (ss + 1)],
                        start=(c == 0),
                        stop=(c == NCC - 1),
                    )
            for ss in range(NJ):
                nc.scalar.activation(
                    GT[:, T * dd + 512 * ss : T * dd + 512 * (ss + 1)],
                    pjs[ss],
                    Ident,
                    bias=g0t[:, dd : dd + 1],
                    scale=1.0,
                )

        # ---- scores (transposed) + query-axis softmax, strip by strip ----
        # L'[s, t] = sum_c' G2T[c', s] * xT[c', t]; strip i = s in [128i, +128)
        pP = ppp.tile([P, OFFS[NT]], bf16)  # exp'd scores (strips)
        for i in range(NT):
            j0 = i // 4
            nt = NJ - j0
            # diagonal tile: columns < d0 lie entirely below the causal
            # boundary; skip them (d0 capped so the moving dim stays >= 256,
            # the full-rate floor for fp32r)
            m = i % 4
            d0 = min(128 * m, 256)
            pls = [psum.tile([P, 512], f32, tag="ps", name=f"pl{i}_{jj}") for jj in range(nt)]
            for cc in range(NCC):
                lhsT = GT[:, T * cc + P * i : T * cc + P * (i + 1)]
                for jj in range(nt):
                    j = j0 + jj
                    lo = d0 if jj == 0 else 0
                    nc.tensor.matmul(
                        pls[jj][:, lo:512],
                        lhsT,
                        xT[:, T * cc + 512 * j + lo : T * cc + 512 * (j + 1)],
                        start=(cc == 0),
                        stop=(cc == NKC - 1),
                    )
            # causal mask on the diagonal tile (additive -1e9 below diagonal)
            nc.vector.tensor_tensor(
                pls[0][:, d0:512],
                pls[0][:, d0:512],
                masks[:, 512 * m + d0 : 512 * (m + 1)],
                ADD,
            )
            # exp(scale * L') -> P' (bf16), with per-tile row sums fused
            parts = stat.tile([P, 4], f32, tag="parts")
            for jj in range(nt):
                lo = d0 if jj == 0 else 0
                nc.scalar.activation(
                    pP[:, OFFS[i] + 512 * jj + lo : OFFS[i] + 512 * (jj + 1)],
                    pls[jj][:, lo:512],
                    Exp,
                    bias=0.0,
                    scale=SCALE,
                    accum_out=parts[:, jj : jj + 1],
                )
            r = stat.tile([P, 1], f32, tag="r")
            nc.vector.reduce_sum(r, parts[:, 0:nt], axis=AX)
            rinv = stat.tile([P, 1], f32, tag="rinv")
            nc.vector.reciprocal(rinv, r)
            # fold softmax denominator into v: vn[s, :] = v[s, :] / r[s]
            nc.vector.tensor_scalar_mul(
                vsb[:, 512 * i : 512 * (i + 1)], vsb[:, 512 * i : 512 * (i + 1)], rinv
            )

        # ---- read[t, v] = sum_s P'[s, t] * vn[s, v] ----
        for u in range(NT):
            pr = psum.tile([P, 512], f32, tag="ps")
            for i in range(u + 1):
                nc.tensor.matmul(
                    pr,
                    pP[:, OFFS[i] + P * u - T0[i] : OFFS[i] + P * (u + 1) - T0[i]],
                    vsb[:, 512 * i : 512 * (i + 1)],
                    start=(i == 0),
                    stop=(i == u),
                )
            ot = outp.tile([P, V], f32, tag="ot")
            nc.vector.tensor_copy(ot, pr)
            nc.sync.dma_start(out_ap[P * u : P * (u + 1), :], ot)


_CACHE = {}


def _build():
    if "nc" in _CACHE:
        return _CACHE["nc"]
    import concourse.bass as bass
    import concourse.tile as tile
    from concourse import bacc, mybir

    f32 = mybir.dt.float32
    f32r = mybir.dt.float32r
    nc = bacc.Bacc("TRN2", target_bir_lowering=False, debug=False)
    x = nc.dram_tensor("x", [T, C], f32r, kind="ExternalInput").ap()
    wq = nc.dram_tensor("wq", [C, K], f32r, kind="ExternalInput").ap()
    bq = nc.dram_tensor("bq", [K], f32, kind="ExternalInput").ap()
    wk = nc.dram_tensor("wk", [C, K], f32r, kind="ExternalInput").ap()
    bk = nc.dram_tensor("bk", [K], f32, kind="ExternalInput").ap()
    wv = nc.dram_tensor("wv", [C, V], f32r, kind="ExternalInput").ap()
    bv = nc.dram_tensor("bv", [V], f32, kind="ExternalInput").ap()
    out = nc.dram_tensor("out", [T, V], f32, kind="ExternalOutput").ap()

    with tile.TileContext(nc) as tc:
        emit(tc, out, x, wq, bq, wk, bk, wv, bv)
    nc.compile()
    _CACHE["nc"] = nc
    return nc


def run_device(x, Wq, bq, Wk, bk, Wv, bv, trace=False):
    """Run the sharded kernel; returns (read [B,T,V], BassKernelResults)."""
    from concourse.bass_utils import run_bass_kernel_spmd

    nc = _build()
    f = np.float32
    base = {
        "wq": np.ascontiguousarray(Wq, f),
        "bq": np.ascontiguousarray(bq, f),
        "wk": np.ascontiguousarray(Wk, f),
        "bk": np.ascontiguousarray(bk, f),
        "wv": np.ascontiguousarray(Wv, f),
        "bv": np.ascontiguousarray(bv, f),
    }
    in_maps = [
        dict(base, x=np.ascontiguousarray(x[b], f)) for b in range(NCORES)
    ]
    res = run_bass_kernel_spmd(
        nc, in_maps, core_ids=list(range(NCORES)), trace=trace
    )
    read = np.stack([res.results[b]["out"] for b in range(NCORES)], axis=0)
    return read, res


def kernel(x, Wq, bq, Wk, bk, Wv, bv):
    x = np.asarray(x, np.float32)
    read, _ = run_device(x, Wq, bq, Wk, bk, Wv, bv, trace=False)
    return np.concatenate((x, read), axis=2)


# revision 23
# speedup vs baseline: 1.0156x; 1.0156x over previous
"""Trainium2 Bass kernel for a causal attention block with softmax over the
QUERY axis (axis=1), data-parallel over batch across 8 NeuronCores.

Reference semantics (per batch element):
    q = x @ Wq + bq ; k = x @ Wk + bk ; v = x @ Wv + bv        # [T, 512]
    logits[t, s] = q[t] . k[s]   masked to s <= t (causal)
    probs = softmax(logits / sqrt(512), axis=t)                # query axis!
    read[t] = sum_s probs[t, s] * v[s]
    out = concat(x, read)

Device kernel computes `read` for one batch element; the batch is sharded
1-per-core across 8 cores and the x-passthrough concat happens on host.

Key layout choice: we materialize the score matrix TRANSPOSED,
L'[s, t] = q[t].k[s], so the softmax reduction (over t) runs along the free
axis, and L' (post-exp) feeds the read matmul directly as the stationary
operand: read[t, v] = sum_s P'[s, t] * vn[s, v] with vn = v / r (the softmax
denominator r[s] is folded into v instead of normalizing the big matrix).
"""

import math
from contextlib import ExitStack

import numpy as np

T = 2048
C = 512  # input channels (contract dim of projections)
K = 512  # key dim (contract dim of logits)
V = 512
P = 128
NCORES = 8
SCALE = 1.0 / math.sqrt(float(K))
NEG = -1.0e9

NT = T // P  # 16 t-chunks (and s-chunks)
NCC = C // P  # 4 contract chunks for projections
NKC = K // P  # 4 contract chunks for logits
NJ = T // 512  # 4 512-wide t slices

# P' strip i covers t in [T0[i], T); strips stored back-to-back (bf16)
T0 = [512 * (i // 4) for i in range(NT)]
WIDTHS = [T - T0[i] for i in range(NT)]
OFFS = np.cumsum([0] + WIDTHS).tolist()  # OFFS[16] == 20480


def emit(tc, out_ap, x, wq, bq, wk, bk, wv, bv):
    import concourse.bass as bass
    import concourse.mybir as mybir
    from concourse.masks import make_identity

    nc = tc.nc
    f32 = mybir.dt.float32
    f32r = mybir.dt.float32r
    bf16 = mybir.dt.bfloat16
    Exp = mybir.ActivationFunctionType.Exp
    Ident = mybir.ActivationFunctionType.Identity
    AX = mybir.AxisListType.X
    ADD = mybir.AluOpType.add

    with ExitStack() as ctx:
        const = ctx.enter_context(tc.tile_pool(name="const", bufs=1))
        # one 4-slot pool of 8KB/partition tiles, time-shared in lifetime
        # order: xg0-3 -> wq, wk -> WqT, WkT -> W2 -> wv
        pa = ctx.enter_context(tc.tile_pool(name="pa", bufs=4))
        xtp = ctx.enter_context(tc.tile_pool(name="xtp", bufs=1))
        gtp = ctx.enter_context(tc.tile_pool(name="gtp", bufs=1))
        ppp = ctx.enter_context(tc.tile_pool(name="ppp", bufs=1))
        vpool = ctx.enter_context(tc.tile_pool(name="vpool", bufs=1))
        stat = ctx.enter_context(tc.tile_pool(name="stat", bufs=4))
        outp = ctx.enter_context(tc.tile_pool(name="outp", bufs=2))
        psum = ctx.enter_context(tc.tile_pool(name="psum", bufs=8, space="PSUM"))

        # ---- input DMAs on the sync HWDGE queue, ordered by need-time ----
        def load_xg(tg):
            xg = pa.tile([P, 4, C], f32r, tag="pa", name=f"xg{tg}")
            nc.sync.dma_start(
                xg, x[512 * tg : 512 * (tg + 1), :].rearrange("(a p) c -> p a c", p=P)
            )
            return xg

        xgs = [load_xg(0)]

        # ---- PE warm-up: dummy bf16 matmuls defeat the HAM cold-start ----
        dummy = const.tile([P, 512], bf16)
        nc.vector.memset(dummy, 0.0)
        dps = psum.tile([P, 512], f32, tag="ps", name="dps")
        for _ in range(8):
            nc.tensor.matmul(dps, dummy[:, 0:P], dummy, start=True, stop=True)

        # wv has a permanent home so its (early-needed) DMA is not slot-gated
        wv_sb = const.tile([P, NCC, 512], f32r)
        for c in range(NCC):
            nc.sync.dma_start(wv_sb[:, c, :], wv[P * c : P * (c + 1), :])
        xgs.append(load_xg(1))
        bv_bc = const.tile([P, V], f32)
        nc.sync.dma_start(
            bv_bc, bass.AP(tensor=bv.tensor, offset=bv.offset, ap=[[0, P], [1, V]])
        )
        bk_bc = const.tile([P, K], f32)
        nc.sync.dma_start(
            bk_bc, bass.AP(tensor=bk.tensor, offset=bk.offset, ap=[[0, P], [1, K]])
        )
        xgs += [load_xg(2), load_xg(3)]

        # weights: f32r in DRAM (bit-identical to f32), per-chunk DMAs.
        # wq/wk land in recycled xg slots (their DMAs wait on the slot WAR);
        # they are only needed by the W2 stage after all four x groups.
        wq_sb = pa.tile([P, NCC, 512], f32r, tag="pa")
        wk_sb = pa.tile([P, NCC, 512], f32r, tag="pa")
        for c in range(NCC):
            nc.sync.dma_start(wq_sb[:, c, :], wq[P * c : P * (c + 1), :])
        for c in range(NCC):
            nc.sync.dma_start(wk_sb[:, c, :], wk[P * c : P * (c + 1), :])

        # ---- constants ----
        identg = const.tile([P, P], f32)
        make_identity(nc, identg)
        ident = const.tile([P, P], f32r)
        nc.vector.tensor_copy(ident, identg)

        # additive causal masks for the diagonal 128x512 tiles; pattern m
        # (m = i mod 4): keep (0.0) where f >= p + 128*m else NEG
        masks = const.tile([P, 4 * 512], f32)
        nc.gpsimd.memset(masks, 0.0)
        for m in range(4):
            sl = masks[:, 512 * m : 512 * (m + 1)]
            nc.gpsimd.affine_select(
                out=sl,
                in_=sl,
                compare_op=mybir.AluOpType.is_ge,
                fill=NEG,
                base=-128 * m,
                channel_multiplier=-1,
                pattern=[[1, 512]],
            )

        def blk_transpose(dst, src_sb, jj):
            """dst[:, jj, :] = 128x512 strip of src^T (4 packed PE transposes)."""
            pt = psum.tile([P, 512], f32r, tag="ps", name="pt")
            for cc in range(4):
                nc.tensor.matmul(
                    pt[:, P * cc : P * (cc + 1)],
                    src_sb[:, cc, P * jj : P * (jj + 1)],
                    ident,
                    is_transpose=True,
                    start=(cc == 0),
                    stop=(cc == 3),
                )
            nc.vector.tensor_copy(dst[:, jj, :], pt)

        # ---- x transposed to xT [c, t] (c on partitions); the bias-product
        # trick below turns the q/k projections into ONE projection, so x is
        # both the moving operand of the scores matmul and the transpose src
        xT = xtp.tile([P, NCC * T], f32r)  # strip c at [:, T*c : T*(c+1)]
        vsb = vpool.tile([P, NT * 512], bf16)  # s-chunk i at [:, 512i:+512]
        for tg in range(4):  # groups of 4 t-chunks
            xg = xgs[tg]
            for c in range(NCC):
                pt = psum.tile([P, 512], f32r, tag="ps")
                for tb in range(4):
                    nc.tensor.matmul(
                        pt[:, P * tb : P * (tb + 1)],
                        xg[:, tb, P * c : P * (c + 1)],
                        ident,
                        is_transpose=True,
                        start=(tb == 0),
                        stop=(tb == 3),
                    )
                nc.scalar.copy(xT[:, T * c + 512 * tg : T * c + 512 * (tg + 1)], pt)
            # transposes don't count as PE-busy for the HAM clock gate
            nc.tensor.matmul(dps, dummy[:, 0:P], dummy, start=True, stop=True)
            # v projection for this group's s-chunks: real PE work that hides
            # the in-flight loads of the later x groups and wq/wk
            for i in range(4 * tg, 4 * tg + 4):
                pv = psum.tile([P, 512], f32, tag="ps")
                for c in range(NCC):
                    nc.tensor.matmul(
                        pv,
                        xT[:, T * c + P * i : T * c + P * (i + 1)],
                        wv_sb[:, c, :],
                        start=(c == 0),
                        stop=(c == NCC - 1),
                    )
                nc.vector.tensor_tensor(vsb[:, 512 * i : 512 * (i + 1)], pv, bv_bc, ADD)

        # Wq^T / Wk^T (W2 = Wk @ Wq^T needs the k-index of both operands on
        # partitions)
        wqT = pa.tile([P, NKC, 512], f32r, tag="pa", name="wqT")
        for jj in range(NKC):
            blk_transpose(wqT, wq_sb, jj)
        nc.tensor.matmul(dps, dummy[:, 0:P], dummy, start=True, stop=True)
        wkT = pa.tile([P, NKC, 512], f32r, tag="pa", name="wkT")
        for jj in range(NKC):
            blk_transpose(wkT, wk_sb, jj)
        nc.tensor.matmul(dps, dummy[:, 0:P], dummy, start=True, stop=True)

        # g0[c'] = sum_j Wq[c', j] * bk[j] (the bk half of the score bias;
        # the bq half is constant along the softmax axis and cancels exactly)
        g0t = stat.tile([P, 4], f32, tag="g0", bufs=1)
        for cc in range(NCC):
            scr = outp.tile([P, 512], f32, tag="ot", name="g0scr")
            nc.vector.tensor_tensor(
                scr, wq_sb[:, cc, :].bitcast(f32), bk_bc, mybir.AluOpType.mult
            )
            nc.vector.reduce_sum(g0t[:, cc : cc + 1], scr, axis=AX)

        # ---- W2 = Wk @ Wq^T  [c, c'] (contract over j on partitions) ----
        w2_sb = pa.tile([P, NCC, 512], f32r, tag="pa", name="w2")
        for a in range(NCC):
            pw = psum.tile([P, 512], f32, tag="ps")
            for jj in range(NKC):
                nc.tensor.matmul(
                    pw,
                    wkT[:, jj, P * a : P * (a + 1)],
                    wqT[:, jj, :],
                    start=(jj == 0),
                    stop=(jj == NKC - 1),
                )
            nc.scalar.copy(w2_sb[:, a, :], pw)

        # ---- G2^T[c', s] = sum_c W2[c, c'] xT[c, s] + g0[c'] ----
        # scores then need NO q/k projections: L'[s,t] = sum_c' G2T[c',s] xT[c',t]
        GT = gtp.tile([P, NCC * T], f32r)  # c'-chunk dd at [:, T*dd:+T]
        for dd in range(NCC):
            pjs = [psum.tile([P, 512], f32, tag="ps", name=f"pg{ss}") for ss in range(NJ)]
            for c in range(NCC):
                lhsT = w2_sb[:, c, P * dd : P * (dd + 1)]
                for ss in range(NJ):
                    nc.tensor.matmul(
                        pjs[ss],
                        lhsT,
                        xT[:, T * c + 512 * ss : T * c + 512 * (ss + 1)],
                        start=(c == 0),
                        stop=(c == NCC - 1),
                    )
            for ss in range(NJ):
                nc.scalar.activation(
                    GT[:, T * dd + 512 * ss : T * dd + 512 * (ss + 1)],
                    pjs[ss],
                    Ident,
                    bias=g0t[:, dd : dd + 1],
                    scale=1.0,
                )

        # ---- scores (transposed) + query-axis softmax, strip by strip ----
        # L'[s, t] = sum_c' G2T[c', s] * xT[c', t]; strip i = s in [128i, +128)
        pP = ppp.tile([P, OFFS[NT]], bf16)  # exp'd scores (strips)
        for i in range(NT):
            j0 = i // 4
            nt = NJ - j0
            # diagonal tile: columns < d0 lie entirely below the causal
            # boundary; skip them (d0 capped so the moving dim stays >= 256,
            # the full-rate floor for fp32r)
            m = i % 4
            d0 = min(128 * m, 256)
            pls = [psum.tile([P, 512], f32, tag="ps", name=f"pl{i}_{jj}") for jj in range(nt)]
            for cc in range(NCC):
                lhsT = GT[:, T * cc + P * i : T * cc + P * (i + 1)]
                for jj in range(nt):
                    j = j0 + jj
                    lo = d0 if jj == 0 else 0
                    nc.tensor.matmul(
                        pls[jj][:, lo:512],
                        lhsT,
                        xT[:, T * cc + 512 * j + lo : T * cc + 512 * (j + 1)],
                        start=(cc == 0),
                        stop=(cc == NKC - 1),
                    )
            # causal mask on the diagonal tile (additive -1e9 below diagonal)
            nc.vector.tensor_tensor(
                pls[0][:, d0:512],
                pls[0][:, d0:512],
                masks[:, 512 * m + d0 : 512 * (m + 1)],
                ADD,
            )
            # exp(scale * L') -> P' (bf16), with per-tile row sums fused
            parts = stat.tile([P, 4], f32, tag="parts")
            for jj in range(nt):
                lo = d0 if jj == 0 else 0
                nc.scalar.activation(
                    pP[:, OFFS[i] + 512 * jj + lo : OFFS[i] + 512 * (jj + 1)],
                    pls[jj][:, lo:512],
                    Exp,
                    bias=0.0,
                    scale=SCALE,
                    accum_out=parts[:, jj : jj + 1],
                )
            r = stat.tile([P, 1], f32, tag="r")
            nc.vector.reduce_sum(r, parts[:, 0:nt], axis=AX)
            rinv = stat.tile([P, 1], f32, tag="rinv")
            nc.vector.reciprocal(rinv, r)
            # fold softmax denominator into v: vn[s, :] = v[s, :] / r[s]
            nc.vector.tensor_scalar_mul(
                vsb[:, 512 * i : 512 * (i + 1)], vsb[:, 512 * i : 512 * (i + 1)], rinv
            )

        # ---- read[t, v] = sum_s P'[s, t] * vn[s, v] ----
        for u in range(NT):
            pr = psum.tile([P, 512], f32, tag="ps")
            for i in range(u + 1):
                nc.tensor.matmul(
                    pr,
                    pP[:, OFFS[i] + P * u - T0[i] : OFFS[i] + P * (u + 1) - T0[i]],
                    vsb[:, 512 * i : 512 * (i + 1)],
                    start=(i == 0),
                    stop=(i == u),
                )
            ot = outp.tile([P, V], f32, tag="ot")
            nc.vector.tensor_copy(ot, pr)
            nc.sync.dma_start(out_ap[P * u : P * (u + 1), :], ot)


_CACHE = {}


def _build():
    if "nc" in _CACHE:
        return _CACHE["nc"]
    import concourse.bass as bass
    import concourse.tile as tile
    from concourse import bacc, mybir

    f32 = mybir.dt.float32
    f32r = mybir.dt.float32r
    nc = bacc.Bacc("TRN2", target_bir_lowering=False, debug=False)
    x = nc.dram_tensor("x", [T, C], f32r, kind="ExternalInput").ap()
    wq = nc.dram_tensor("wq", [C, K], f32r, kind="ExternalInput").ap()
    bq = nc.dram_tensor("bq", [K], f32, kind="ExternalInput").ap()
    wk = nc.dram_tensor("wk", [C, K], f32r, kind="ExternalInput").ap()
    bk = nc.dram_tensor("bk", [K], f32, kind="ExternalInput").ap()
    wv = nc.dram_tensor("wv", [C, V], f32r, kind="ExternalInput").ap()
    bv = nc.dram_tensor("bv", [V], f32, kind="ExternalInput").ap()
    out = nc.dram_tensor("out", [T, V], f32, kind="ExternalOutput").ap()

    with tile.TileContext(nc) as tc:
        emit(tc, out, x, wq, bq, wk, bk, wv, bv)
    nc.compile()
    _CACHE["nc"] = nc
    return nc


def run_device(x, Wq, bq, Wk, bk, Wv, bv, trace=False):
    """Run the sharded kernel; returns (read [B,T,V], BassKernelResults)."""
    from concourse.bass_utils import run_bass_kernel_spmd

    nc = _build()
    f = np.float32
    base = {
        "wq": np.ascontiguousarray(Wq, f),
        "bq": np.ascontiguousarray(bq, f),
        "wk": np.ascontiguousarray(Wk, f),
        "bk": np.ascontiguousarray(bk, f),
        "wv": np.ascontiguousarray(Wv, f),
        "bv": np.ascontiguousarray(bv, f),
    }
    in_maps = [
        dict(base, x=np.ascontiguousarray(x[b], f)) for b in range(NCORES)
    ]
    res = run_bass_kernel_spmd(
        nc, in_maps, core_ids=list(range(NCORES)), trace=trace
    )
    read = np.stack([res.results[b]["out"] for b in range(NCORES)], axis=0)
    return read, res


def kernel(x, Wq, bq, Wk, bk, Wv, bv):
    x = np.asarray(x, np.float32)
    read, _ = run_device(x, Wq, bq, Wk, bk, Wv, bv, trace=False)
    return np.concatenate((x, read), axis=2)


# revision 24
# speedup vs baseline: 1.1797x; 1.1616x over previous
"""Trainium2 Bass kernel for a causal attention block with softmax over the
QUERY axis (axis=1), data-parallel over batch across 8 NeuronCores.

Reference semantics (per batch element):
    q = x @ Wq + bq ; k = x @ Wk + bk ; v = x @ Wv + bv        # [T, 512]
    logits[t, s] = q[t] . k[s]   masked to s <= t (causal)
    probs = softmax(logits / sqrt(512), axis=t)                # query axis!
    read[t] = sum_s probs[t, s] * v[s]
    out = concat(x, read)

Device kernel computes `read` for one batch element; the batch is sharded
1-per-core across 8 cores and the x-passthrough concat happens on host.

Key layout choice: we materialize the score matrix TRANSPOSED,
L'[s, t] = q[t].k[s], so the softmax reduction (over t) runs along the free
axis, and L' (post-exp) feeds the read matmul directly as the stationary
operand: read[t, v] = sum_s P'[s, t] * vn[s, v] with vn = v / r (the softmax
denominator r[s] is folded into v instead of normalizing the big matrix).
"""

import math
from contextlib import ExitStack

import numpy as np

T = 2048
C = 512  # input channels (contract dim of projections)
K = 512  # key dim (contract dim of logits)
V = 512
P = 128
NCORES = 8
SCALE = 1.0 / math.sqrt(float(K))
NEG = -1.0e9

NT = T // P  # 16 t-chunks (and s-chunks)
NCC = C // P  # 4 contract chunks for projections
NKC = K // P  # 4 contract chunks for logits
NJ = T // 512  # 4 512-wide t slices

# P' strip i covers t in [T0[i], T); strips stored back-to-back (bf16)
T0 = [512 * (i // 4) for i in range(NT)]
WIDTHS = [T - T0[i] for i in range(NT)]
OFFS = np.cumsum([0] + WIDTHS).tolist()  # OFFS[16] == 20480


def emit(tc, out_ap, x, wq, bq, wk, bk, wv, bv):
    import concourse.bass as bass
    import concourse.mybir as mybir
    from concourse.masks import make_identity

    nc = tc.nc
    f32 = mybir.dt.float32
    f32r = mybir.dt.float32r
    bf16 = mybir.dt.bfloat16
    Exp = mybir.ActivationFunctionType.Exp
    Ident = mybir.ActivationFunctionType.Identity
    AX = mybir.AxisListType.X
    ADD = mybir.AluOpType.add

    with ExitStack() as ctx:
        const = ctx.enter_context(tc.tile_pool(name="const", bufs=1))
        # one 4-slot pool of 8KB/partition tiles, time-shared in lifetime
        # order: xg0-3 -> wq, wk -> WqT, WkT -> W2 -> wv
        pa = ctx.enter_context(tc.tile_pool(name="pa", bufs=4))
        xtp = ctx.enter_context(tc.tile_pool(name="xtp", bufs=1))
        gtp = ctx.enter_context(tc.tile_pool(name="gtp", bufs=1))
        ppp = ctx.enter_context(tc.tile_pool(name="ppp", bufs=1))
        vpool = ctx.enter_context(tc.tile_pool(name="vpool", bufs=1))
        stat = ctx.enter_context(tc.tile_pool(name="stat", bufs=4))
        outp = ctx.enter_context(tc.tile_pool(name="outp", bufs=2))
        psum = ctx.enter_context(tc.tile_pool(name="psum", bufs=8, space="PSUM"))

        # ---- input DMAs on the sync HWDGE queue, ordered by need-time ----
        def load_xg(tg):
            xg = pa.tile([P, 4, C], f32r, tag="pa", name=f"xg{tg}")
            nc.sync.dma_start(
                xg, x[512 * tg : 512 * (tg + 1), :].rearrange("(a p) c -> p a c", p=P)
            )
            return xg

        xgs = [load_xg(0)]

        # ---- PE warm-up: dummy bf16 matmuls defeat the HAM cold-start ----
        dummy = const.tile([P, 512], bf16)
        nc.vector.memset(dummy, 0.0)
        dps = psum.tile([P, 512], f32, tag="ps", name="dps")
        for _ in range(8):
            nc.tensor.matmul(dps, dummy[:, 0:P], dummy, start=True, stop=True)

        # wv has a permanent home so its (early-needed) DMA is not slot-gated
        wv_sb = const.tile([P, NCC, 512], f32r)
        for c in range(NCC):
            nc.sync.dma_start(wv_sb[:, c, :], wv[P * c : P * (c + 1), :])
        xgs.append(load_xg(1))
        bv_bc = const.tile([P, V], f32)
        nc.sync.dma_start(
            bv_bc, bass.AP(tensor=bv.tensor, offset=bv.offset, ap=[[0, P], [1, V]])
        )
        bk_bc = const.tile([P, K], f32)
        nc.sync.dma_start(
            bk_bc, bass.AP(tensor=bk.tensor, offset=bk.offset, ap=[[0, P], [1, K]])
        )
        xgs += [load_xg(2), load_xg(3)]

        # weights: f32r in DRAM (bit-identical to f32), per-chunk DMAs.
        # wq/wk land in recycled xg slots (their DMAs wait on the slot WAR);
        # they are only needed by the W2 stage after all four x groups.
        wq_sb = pa.tile([P, NCC, 512], f32r, tag="pa")
        wk_sb = pa.tile([P, NCC, 512], f32r, tag="pa")
        for c in range(NCC):
            nc.sync.dma_start(wq_sb[:, c, :], wq[P * c : P * (c + 1), :])
        for c in range(NCC):
            nc.sync.dma_start(wk_sb[:, c, :], wk[P * c : P * (c + 1), :])

        # ---- constants ----
        identg = const.tile([P, P], f32)
        make_identity(nc, identg)
        ident = const.tile([P, P], f32r)
        nc.vector.tensor_copy(ident, identg)

        # additive causal masks for the diagonal 128x512 tiles; pattern m
        # (m = i mod 4): keep (0.0) where f >= p + 128*m else NEG
        masks = const.tile([P, 4 * 512], f32)
        nc.gpsimd.memset(masks, 0.0)
        for m in range(4):
            sl = masks[:, 512 * m : 512 * (m + 1)]
            nc.gpsimd.affine_select(
                out=sl,
                in_=sl,
                compare_op=mybir.AluOpType.is_ge,
                fill=NEG,
                base=-128 * m,
                channel_multiplier=-1,
                pattern=[[1, 512]],
            )

        def blk_transpose(dst, src_sb, jj):
            """dst[:, jj, :] = 128x512 strip of src^T (4 packed PE transposes)."""
            pt = psum.tile([P, 512], f32r, tag="ps", name="pt")
            for cc in range(4):
                nc.tensor.matmul(
                    pt[:, P * cc : P * (cc + 1)],
                    src_sb[:, cc, P * jj : P * (jj + 1)],
                    ident,
                    is_transpose=True,
                    start=(cc == 0),
                    stop=(cc == 3),
                )
            nc.vector.tensor_copy(dst[:, jj, :], pt)

        # ---- x transposed to xT [c, t] (c on partitions); the bias-product
        # trick below turns the q/k projections into ONE projection, so x is
        # both the moving operand of the scores matmul and the transpose src
        xT = xtp.tile([P, NCC * T], f32r)  # strip c at [:, T*c : T*(c+1)]
        vsb = vpool.tile([P, NT * 512], bf16)  # s-chunk i at [:, 512i:+512]
        for tg in range(4):  # groups of 4 t-chunks
            xg = xgs[tg]
            for c in range(NCC):
                pt = psum.tile([P, 512], f32r, tag="ps")
                for tb in range(4):
                    nc.tensor.matmul(
                        pt[:, P * tb : P * (tb + 1)],
                        xg[:, tb, P * c : P * (c + 1)],
                        ident,
                        is_transpose=True,
                        start=(tb == 0),
                        stop=(tb == 3),
                    )
                nc.scalar.copy(xT[:, T * c + 512 * tg : T * c + 512 * (tg + 1)], pt)
            # v projection for this group's s-chunks: real PE work that hides
            # the in-flight loads of the later x groups and wq/wk
            for i in range(4 * tg, 4 * tg + 4):
                pv = psum.tile([P, 512], f32, tag="ps")
                for c in range(NCC):
                    nc.tensor.matmul(
                        pv,
                        xT[:, T * c + P * i : T * c + P * (i + 1)],
                        wv_sb[:, c, :],
                        start=(c == 0),
                        stop=(c == NCC - 1),
                    )
                nc.vector.tensor_tensor(vsb[:, 512 * i : 512 * (i + 1)], pv, bv_bc, ADD)

        # Wq^T / Wk^T (W2 = Wk @ Wq^T needs the k-index of both operands on
        # partitions)
        wqT = pa.tile([P, NKC, 512], f32r, tag="pa", name="wqT")
        for jj in range(NKC):
            blk_transpose(wqT, wq_sb, jj)
        nc.tensor.matmul(dps, dummy[:, 0:P], dummy, start=True, stop=True)
        wkT = pa.tile([P, NKC, 512], f32r, tag="pa", name="wkT")
        for jj in range(NKC):
            blk_transpose(wkT, wk_sb, jj)

        # g0[c'] = sum_j Wq[c', j] * bk[j] (the bk half of the score bias;
        # the bq half is constant along the softmax axis and cancels exactly)
        g0t = stat.tile([P, 4], f32, tag="g0", bufs=1)
        for cc in range(NCC):
            scr = outp.tile([P, 512], f32, tag="ot", name="g0scr")
            nc.vector.tensor_tensor(
                scr, wq_sb[:, cc, :].bitcast(f32), bk_bc, mybir.AluOpType.mult
            )
            nc.vector.reduce_sum(g0t[:, cc : cc + 1], scr, axis=AX)

        # ---- W2 = Wk @ Wq^T  [c, c'] (contract over j on partitions) ----
        w2_sb = pa.tile([P, NCC, 512], f32r, tag="pa", name="w2")
        for a in range(NCC):
            pw = psum.tile([P, 512], f32, tag="ps")
            for jj in range(NKC):
                nc.tensor.matmul(
                    pw,
                    wkT[:, jj, P * a : P * (a + 1)],
                    wqT[:, jj, :],
                    start=(jj == 0),
                    stop=(jj == NKC - 1),
                )
            nc.vector.tensor_copy(w2_sb[:, a, :], pw)

        # ---- G2^T[c', s] = sum_c W2[c, c'] xT[c, s] + g0[c'] ----
        # scores then need NO q/k projections: L'[s,t] = sum_c' G2T[c',s] xT[c',t]
        GT = gtp.tile([P, NCC * T], f32r)  # c'-chunk dd at [:, T*dd:+T]
        for dd in range(NCC):
            pjs = [psum.tile([P, 512], f32, tag="ps", name=f"pg{ss}") for ss in range(NJ)]
            for c in range(NCC):
                lhsT = w2_sb[:, c, P * dd : P * (dd + 1)]
                for ss in range(NJ):
                    nc.tensor.matmul(
                        pjs[ss],
                        lhsT,
                        xT[:, T * c + 512 * ss : T * c + 512 * (ss + 1)],
                        start=(c == 0),
                        stop=(c == NCC - 1),
                    )
            for ss in range(NJ):
                nc.scalar.activation(
                    GT[:, T * dd + 512 * ss : T * dd + 512 * (ss + 1)],
                    pjs[ss],
                    Ident,
                    bias=g0t[:, dd : dd + 1],
                    scale=1.0,
                )

        # ---- scores (transposed) + query-axis softmax, strip by strip ----
        # L'[s, t] = sum_c' G2T[c', s] * xT[c', t]; strip i = s in [128i, +128)
        pP = ppp.tile([P, OFFS[NT]], bf16)  # exp'd scores (strips)
        for i in range(NT):
            j0 = i // 4
            nt = NJ - j0
            # diagonal tile: columns < d0 lie entirely below the causal
            # boundary; skip them (d0 capped so the moving dim stays >= 256,
            # the full-rate floor for fp32r)
            m = i % 4
            d0 = min(128 * m, 256)
            pls = [psum.tile([P, 512], f32, tag="ps", name=f"pl{i}_{jj}") for jj in range(nt)]
            for cc in range(NCC):
                lhsT = GT[:, T * cc + P * i : T * cc + P * (i + 1)]
                for jj in range(nt):
                    j = j0 + jj
                    lo = d0 if jj == 0 else 0
                    nc.tensor.matmul(
                        pls[jj][:, lo:512],
                        lhsT,
                        xT[:, T * cc + 512 * j + lo : T * cc + 512 * (j + 1)],
                        start=(cc == 0),
                        stop=(cc == NKC - 1),
                    )
            # causal mask on the diagonal tile (additive -1e9 below diagonal)
            nc.vector.tensor_tensor(
                pls[0][:, d0:512],
                pls[0][:, d0:512],
                masks[:, 512 * m + d0 : 512 * (m + 1)],
                ADD,
            )
            # exp(scale * L') -> P' (bf16), with per-tile row sums fused
            parts = stat.tile([P, 4], f32, tag="parts")
            for jj in range(nt):
                lo = d0 if jj == 0 else 0
                nc.scalar.activation(
                    pP[:, OFFS[i] + 512 * jj + lo : OFFS[i] + 512 * (jj + 1)],
                    pls[jj][:, lo:512],
                    Exp,
                    bias=0.0,
                    scale=SCALE,
                    accum_out=parts[:, jj : jj + 1],
                )
            r = stat.tile([P, 1], f32, tag="r")
            nc.vector.reduce_sum(r, parts[:, 0:nt], axis=AX)
            rinv = stat.tile([P, 1], f32, tag="rinv")
            nc.vector.reciprocal(rinv, r)
            # fold softmax denominator into v: vn[s, :] = v[s, :] / r[s]
            nc.vector.tensor_scalar_mul(
                vsb[:, 512 * i : 512 * (i + 1)], vsb[:, 512 * i : 512 * (i + 1)], rinv
            )

        # ---- read[t, v] = sum_s P'[s, t] * vn[s, v] ----
        for u in range(NT):
            pr = psum.tile([P, 512], f32, tag="ps")
            for i in range(u + 1):
                nc.tensor.matmul(
                    pr,
                    pP[:, OFFS[i] + P * u - T0[i] : OFFS[i] + P * (u + 1) - T0[i]],
                    vsb[:, 512 * i : 512 * (i + 1)],
                    start=(i == 0),
                    stop=(i == u),
                )
            ot = outp.tile([P, V], f32, tag="ot")
            nc.vector.tensor_copy(ot, pr)
            nc.sync.dma_start(out_ap[P * u : P * (u + 1), :], ot)


_CACHE = {}


def _build():
    if "nc" in _CACHE:
        return _CACHE["nc"]
    import concourse.bass as bass
    import concourse.tile as tile
    from concourse import bacc, mybir

    f32 = mybir.dt.float32
    f32r = mybir.dt.float32r
    nc = bacc.Bacc("TRN2", target_bir_lowering=False, debug=False)
    x = nc.dram_tensor("x", [T, C], f32r, kind="ExternalInput").ap()
    wq = nc.dram_tensor("wq", [C, K], f32r, kind="ExternalInput").ap()
    bq = nc.dram_tensor("bq", [K], f32, kind="ExternalInput").ap()
    wk = nc.dram_tensor("wk", [C, K], f32r, kind="ExternalInput").ap()
    bk = nc.dram_tensor("bk", [K], f32, kind="ExternalInput").ap()
    wv = nc.dram_tensor("wv", [C, V], f32r, kind="ExternalInput").ap()
    bv = nc.dram_tensor("bv", [V], f32, kind="ExternalInput").ap()
    out = nc.dram_tensor("out", [T, V], f32, kind="ExternalOutput").ap()

    with tile.TileContext(nc) as tc:
        emit(tc, out, x, wq, bq, wk, bk, wv, bv)
    nc.compile()
    _CACHE["nc"] = nc
    return nc


def run_device(x, Wq, bq, Wk, bk, Wv, bv, trace=False):
    """Run the sharded kernel; returns (read [B,T,V], BassKernelResults)."""
    from concourse.bass_utils import run_bass_kernel_spmd

    nc = _build()
    f = np.float32
    base = {
        "wq": np.ascontiguousarray(Wq, f),
        "bq": np.ascontiguousarray(bq, f),
        "wk": np.ascontiguousarray(Wk, f),
        "bk": np.ascontiguousarray(bk, f),
        "wv": np.ascontiguousarray(Wv, f),
        "bv": np.ascontiguousarray(bv, f),
    }
    in_maps = [
        dict(base, x=np.ascontiguousarray(x[b], f)) for b in range(NCORES)
    ]
    res = run_bass_kernel_spmd(
        nc, in_maps, core_ids=list(range(NCORES)), trace=trace
    )
    read = np.stack([res.results[b]["out"] for b in range(NCORES)], axis=0)
    return read, res


def kernel(x, Wq, bq, Wk, bk, Wv, bv):
    x = np.asarray(x, np.float32)
    read, _ = run_device(x, Wq, bq, Wk, bk, Wv, bv, trace=False)
    return np.concatenate((x, read), axis=2)


# revision 25
# speedup vs baseline: 1.2064x; 1.0226x over previous
"""Trainium2 Bass kernel for a causal attention block with softmax over the
QUERY axis (axis=1), data-parallel over batch across 8 NeuronCores.

Reference semantics (per batch element):
    q = x @ Wq + bq ; k = x @ Wk + bk ; v = x @ Wv + bv        # [T, 512]
    logits[t, s] = q[t] . k[s]   masked to s <= t (causal)
    probs = softmax(logits / sqrt(512), axis=t)                # query axis!
    read[t] = sum_s probs[t, s] * v[s]
    out = concat(x, read)

Device kernel computes `read` for one batch element; the batch is sharded
1-per-core across 8 cores and the x-passthrough concat happens on host.

Key layout choice: we materialize the score matrix TRANSPOSED,
L'[s, t] = q[t].k[s], so the softmax reduction (over t) runs along the free
axis, and L' (post-exp) feeds the read matmul directly as the stationary
operand: read[t, v] = sum_s P'[s, t] * vn[s, v] with vn = v / r (the softmax
denominator r[s] is folded into v instead of normalizing the big matrix).
"""

import math
from contextlib import ExitStack

import numpy as np

T = 2048
C = 512  # input channels (contract dim of projections)
K = 512  # key dim (contract dim of logits)
V = 512
P = 128
NCORES = 8
SCALE = 1.0 / math.sqrt(float(K))
NEG = -1.0e9

NT = T // P  # 16 t-chunks (and s-chunks)
NCC = C // P  # 4 contract chunks for projections
NKC = K // P  # 4 contract chunks for logits
NJ = T // 512  # 4 512-wide t slices

# P' strip i covers t in [T0[i], T); strips stored back-to-back (bf16)
T0 = [512 * (i // 4) for i in range(NT)]
WIDTHS = [T - T0[i] for i in range(NT)]
OFFS = np.cumsum([0] + WIDTHS).tolist()  # OFFS[16] == 20480


def emit(tc, out_ap, x, wq, bq, wk, bk, wv, bv):
    import concourse.bass as bass
    import concourse.mybir as mybir
    from concourse.masks import make_identity

    nc = tc.nc
    f32 = mybir.dt.float32
    f32r = mybir.dt.float32r
    bf16 = mybir.dt.bfloat16
    Exp = mybir.ActivationFunctionType.Exp
    Ident = mybir.ActivationFunctionType.Identity
    AX = mybir.AxisListType.X
    ADD = mybir.AluOpType.add

    with ExitStack() as ctx:
        const = ctx.enter_context(tc.tile_pool(name="const", bufs=1))
        # one 4-slot pool of 8KB/partition tiles, time-shared in lifetime
        # order: xg0-3 -> wq, wk -> WqT, WkT -> W2 -> wv
        pa = ctx.enter_context(tc.tile_pool(name="pa", bufs=4))
        xtp = ctx.enter_context(tc.tile_pool(name="xtp", bufs=1))
        gtp = ctx.enter_context(tc.tile_pool(name="gtp", bufs=1))
        ppp = ctx.enter_context(tc.tile_pool(name="ppp", bufs=1))
        vpool = ctx.enter_context(tc.tile_pool(name="vpool", bufs=1))
        stat = ctx.enter_context(tc.tile_pool(name="stat", bufs=4))
        outp = ctx.enter_context(tc.tile_pool(name="outp", bufs=2))
        psum = ctx.enter_context(tc.tile_pool(name="psum", bufs=8, space="PSUM"))

        # ---- input DMAs on the sync HWDGE queue, ordered by need-time ----
        def load_xg(tg):
            xg = pa.tile([P, 4, C], f32r, tag="pa", name=f"xg{tg}")
            nc.sync.dma_start(
                xg, x[512 * tg : 512 * (tg + 1), :].rearrange("(a p) c -> p a c", p=P)
            )
            return xg

        xgs = [load_xg(0)]

        # ---- PE warm-up: dummy bf16 matmuls defeat the HAM cold-start ----
        dummy = const.tile([P, 512], bf16)
        nc.vector.memset(dummy, 0.0)
        dps = psum.tile([P, 512], f32, tag="ps", name="dps")
        for _ in range(8):
            nc.tensor.matmul(dps, dummy[:, 0:P], dummy, start=True, stop=True)

        # wv has a permanent home so its (early-needed) DMA is not slot-gated
        wv_sb = const.tile([P, NCC, 512], f32r)
        for c in range(NCC):
            nc.sync.dma_start(wv_sb[:, c, :], wv[P * c : P * (c + 1), :])
        xgs.append(load_xg(1))
        bv_bc = const.tile([P, V], f32)
        nc.sync.dma_start(
            bv_bc, bass.AP(tensor=bv.tensor, offset=bv.offset, ap=[[0, P], [1, V]])
        )
        bk_bc = const.tile([P, K], f32)
        nc.sync.dma_start(
            bk_bc, bass.AP(tensor=bk.tensor, offset=bk.offset, ap=[[0, P], [1, K]])
        )
        xgs += [load_xg(2), load_xg(3)]

        # weights: f32r in DRAM (bit-identical to f32), per-chunk DMAs.
        # wq/wk land in recycled xg slots (their DMAs wait on the slot WAR);
        # they are only needed by the W2 stage after all four x groups.
        wq_sb = pa.tile([P, NCC, 512], f32r, tag="pa")
        wk_sb = pa.tile([P, NCC, 512], f32r, tag="pa")
        for c in range(NCC):
            nc.sync.dma_start(wq_sb[:, c, :], wq[P * c : P * (c + 1), :])
        for c in range(NCC):
            nc.sync.dma_start(wk_sb[:, c, :], wk[P * c : P * (c + 1), :])

        # ---- constants ----
        identg = const.tile([P, P], f32)
        make_identity(nc, identg)
        ident = const.tile([P, P], f32r)
        nc.vector.tensor_copy(ident, identg)

        # additive causal masks for the diagonal 128x512 tiles; pattern m
        # (m = i mod 4): keep (0.0) where f >= p + 128*m else NEG
        masks = const.tile([P, 4 * 512], f32)
        nc.gpsimd.memset(masks, 0.0)
        for m in range(4):
            sl = masks[:, 512 * m : 512 * (m + 1)]
            nc.gpsimd.affine_select(
                out=sl,
                in_=sl,
                compare_op=mybir.AluOpType.is_ge,
                fill=NEG,
                base=-128 * m,
                channel_multiplier=-1,
                pattern=[[1, 512]],
            )

        def blk_transpose(dst, src_sb, jj):
            """dst[:, jj, :] = 128x512 strip of src^T (4 packed PE transposes)."""
            pt = psum.tile([P, 512], f32r, tag="ps", name="pt")
            for cc in range(4):
                nc.tensor.matmul(
                    pt[:, P * cc : P * (cc + 1)],
                    src_sb[:, cc, P * jj : P * (jj + 1)],
                    ident,
                    is_transpose=True,
                    start=(cc == 0),
                    stop=(cc == 3),
                )
            nc.vector.tensor_copy(dst[:, jj, :], pt)

        # ---- x transposed to xT [c, t] (c on partitions); the bias-product
        # trick below turns the q/k projections into ONE projection, so x is
        # both the moving operand of the scores matmul and the transpose src
        xT = xtp.tile([P, NCC * T], f32r)  # strip c at [:, T*c : T*(c+1)]
        vsb = vpool.tile([P, NT * 512], bf16)  # s-chunk i at [:, 512i:+512]
        for tg in range(4):  # groups of 4 t-chunks
            xg = xgs[tg]
            for c in range(NCC):
                pt = psum.tile([P, 512], f32r, tag="ps")
                for tb in range(4):
                    nc.tensor.matmul(
                        pt[:, P * tb : P * (tb + 1)],
                        xg[:, tb, P * c : P * (c + 1)],
                        ident,
                        is_transpose=True,
                        start=(tb == 0),
                        stop=(tb == 3),
                    )
                nc.scalar.copy(xT[:, T * c + 512 * tg : T * c + 512 * (tg + 1)], pt)
            # transposes don't count as PE-busy for the HAM clock gate
            nc.tensor.matmul(dps, dummy[:, 0:P], dummy, start=True, stop=True)
            # v projection for this group's s-chunks: real PE work that hides
            # the in-flight loads of the later x groups and wq/wk
            for i in range(4 * tg, 4 * tg + 4):
                pv = psum.tile([P, 512], f32, tag="ps")
                for c in range(NCC):
                    nc.tensor.matmul(
                        pv,
                        xT[:, T * c + P * i : T * c + P * (i + 1)],
                        wv_sb[:, c, :],
                        start=(c == 0),
                        stop=(c == NCC - 1),
                    )
                nc.vector.tensor_tensor(vsb[:, 512 * i : 512 * (i + 1)], pv, bv_bc, ADD)

        # Wq^T / Wk^T (W2 = Wk @ Wq^T needs the k-index of both operands on
        # partitions)
        wqT = pa.tile([P, NKC, 512], f32r, tag="pa", name="wqT")
        for jj in range(NKC):
            blk_transpose(wqT, wq_sb, jj)
        nc.tensor.matmul(dps, dummy[:, 0:P], dummy, start=True, stop=True)
        wkT = pa.tile([P, NKC, 512], f32r, tag="pa", name="wkT")
        for jj in range(NKC):
            blk_transpose(wkT, wk_sb, jj)
        nc.tensor.matmul(dps, dummy[:, 0:P], dummy, start=True, stop=True)

        # g0[c'] = sum_j Wq[c', j] * bk[j] (the bk half of the score bias;
        # the bq half is constant along the softmax axis and cancels exactly)
        g0t = stat.tile([P, 4], f32, tag="g0", bufs=1)
        for cc in range(NCC):
            scr = outp.tile([P, 512], f32, tag="ot", name="g0scr")
            nc.vector.tensor_tensor(
                scr, wq_sb[:, cc, :].bitcast(f32), bk_bc, mybir.AluOpType.mult
            )
            nc.vector.reduce_sum(g0t[:, cc : cc + 1], scr, axis=AX)

        # ---- W2 = Wk @ Wq^T  [c, c'] (contract over j on partitions) ----
        w2_sb = pa.tile([P, NCC, 512], f32r, tag="pa", name="w2")
        for a in range(NCC):
            pw = psum.tile([P, 512], f32, tag="ps")
            for jj in range(NKC):
                nc.tensor.matmul(
                    pw,
                    wkT[:, jj, P * a : P * (a + 1)],
                    wqT[:, jj, :],
                    start=(jj == 0),
                    stop=(jj == NKC - 1),
                )
            nc.scalar.copy(w2_sb[:, a, :], pw)

        # ---- G2^T[c', s] = sum_c W2[c, c'] xT[c, s] + g0[c'] ----
        # scores then need NO q/k projections: L'[s,t] = sum_c' G2T[c',s] xT[c',t]
        GT = gtp.tile([P, NCC * T], f32r)  # c'-chunk dd at [:, T*dd:+T]
        for dd in range(NCC):
            pjs = [psum.tile([P, 512], f32, tag="ps", name=f"pg{ss}") for ss in range(NJ)]
            for c in range(NCC):
                lhsT = w2_sb[:, c, P * dd : P * (dd + 1)]
                for ss in range(NJ):
                    nc.tensor.matmul(
                        pjs[ss],
                        lhsT,
                        xT[:, T * c + 512 * ss : T * c + 512 * (ss + 1)],
                        start=(c == 0),
                        stop=(c == NCC - 1),
                    )
            for ss in range(NJ):
                nc.scalar.activation(
                    GT[:, T * dd + 512 * ss : T * dd + 512 * (ss + 1)],
                    pjs[ss],
                    Ident,
                    bias=g0t[:, dd : dd + 1],
                    scale=1.0,
                )

        # ---- scores (transposed) + query-axis softmax, strip by strip ----
        # L'[s, t] = sum_c' G2T[c', s] * xT[c', t]; strip i = s in [128i, +128)
        pP = ppp.tile([P, OFFS[NT]], bf16)  # exp'd scores (strips)
        for i in range(NT):
            j0 = i // 4
            nt = NJ - j0
            # diagonal tile: columns < d0 lie entirely below the causal
            # boundary; skip them (d0 capped so the moving dim stays >= 256,
            # the full-rate floor for fp32r)
            m = i % 4
            d0 = min(128 * m, 256)
            pls = [psum.tile([P, 512], f32, tag="ps", name=f"pl{i}_{jj}") for jj in range(nt)]
            for cc in range(NCC):
                lhsT = GT[:, T * cc + P * i : T * cc + P * (i + 1)]
                for jj in range(nt):
                    j = j0 + jj
                    lo = d0 if jj == 0 else 0
                    nc.tensor.matmul(
                        pls[jj][:, lo:512],
                        lhsT,
                        xT[:, T * cc + 512 * j + lo : T * cc + 512 * (j + 1)],
                        start=(cc == 0),
                        stop=(cc == NKC - 1),
                    )
            # causal mask on the diagonal tile (additive -1e9 below diagonal)
            nc.vector.tensor_tensor(
                pls[0][:, d0:512],
                pls[0][:, d0:512],
                masks[:, 512 * m + d0 : 512 * (m + 1)],
                ADD,
            )
            # exp(scale * L') -> P' (bf16), with per-tile row sums fused
            parts = stat.tile([P, 4], f32, tag="parts")
            for jj in range(nt):
                lo = d0 if jj == 0 else 0
                nc.scalar.activation(
                    pP[:, OFFS[i] + 512 * jj + lo : OFFS[i] + 512 * (jj + 1)],
                    pls[jj][:, lo:512],
                    Exp,
                    bias=0.0,
                    scale=SCALE,
                    accum_out=parts[:, jj : jj + 1],
                )
            r = stat.tile([P, 1], f32, tag="r")
            nc.vector.reduce_sum(r, parts[:, 0:nt], axis=AX)
            rinv = stat.tile([P, 1], f32, tag="rinv")
            nc.vector.reciprocal(rinv, r)
            # fold softmax denominator into v: vn[s, :] = v[s, :] / r[s]
            nc.vector.tensor_scalar_mul(
                vsb[:, 512 * i : 512 * (i + 1)], vsb[:, 512 * i : 512 * (i + 1)], rinv
            )

        # ---- read[t, v] = sum_s P'[s, t] * vn[s, v] ----
        for u in range(NT):
            pr = psum.tile([P, 512], f32, tag="ps")
            for i in range(u + 1):
                nc.tensor.matmul(
                    pr,
                    pP[:, OFFS[i] + P * u - T0[i] : OFFS[i] + P * (u + 1) - T0[i]],
                    vsb[:, 512 * i : 512 * (i + 1)],
                    start=(i == 0),
                    stop=(i == u),
                )
            ot = outp.tile([P, V], f32, tag="ot")
            nc.vector.tensor_copy(ot, pr)
            nc.sync.dma_start(out_ap[P * u : P * (u + 1), :], ot)


_CACHE = {}


def _build():
    if "nc" in _CACHE:
        return _CACHE["nc"]
    import concourse.bass as bass
    import concourse.tile as tile
    from concourse import bacc, mybir

    f32 = mybir.dt.float32
    f32r = mybir.dt.float32r
    nc = bacc.Bacc("TRN2", target_bir_lowering=False, debug=False)
    x = nc.dram_tensor("x", [T, C], f32r, kind="ExternalInput").ap()
    wq = nc.dram_tensor("wq", [C, K], f32r, kind="ExternalInput").ap()
    bq = nc.dram_tensor("bq", [K], f32, kind="ExternalInput").ap()
    wk = nc.dram_tensor("wk", [C, K], f32r, kind="ExternalInput").ap()
    bk = nc.dram_tensor("bk", [K], f32, kind="ExternalInput").ap()
    wv = nc.dram_tensor("wv", [C, V], f32r, kind="ExternalInput").ap()
    bv = nc.dram_tensor("bv", [V], f32, kind="ExternalInput").ap()
    out = nc.dram_tensor("out", [T, V], f32, kind="ExternalOutput").ap()

    with tile.TileContext(nc) as tc:
        emit(tc, out, x, wq, bq, wk, bk, wv, bv)
    nc.compile()
    _CACHE["nc"] = nc
    return nc


def run_device(x, Wq, bq, Wk, bk, Wv, bv, trace=False):
    """Run the sharded kernel; returns (read [B,T,V], BassKernelResults)."""
    from concourse.bass_utils import run_bass_kernel_spmd

    nc = _build()
    f = np.float32
    base = {
        "wq": np.ascontiguousarray(Wq, f),
        "bq": np.ascontiguousarray(bq, f),
        "wk": np.ascontiguousarray(Wk, f),
        "bk": np.ascontiguousarray(bk, f),
        "wv": np.ascontiguousarray(Wv, f),
        "bv": np.ascontiguousarray(bv, f),
    }
    in_maps = [
        dict(base, x=np.ascontiguousarray(x[b], f)) for b in range(NCORES)
    ]
    res = run_bass_kernel_spmd(
        nc, in_maps, core_ids=list(range(NCORES)), trace=trace
    )
    read = np.stack([res.results[b]["out"] for b in range(NCORES)], axis=0)
    return read, res


def kernel(x, Wq, bq, Wk, bk, Wv, bv):
    x = np.asarray(x, np.float32)
    read, _ = run_device(x, Wq, bq, Wk, bk, Wv, bv, trace=False)
    return np.concatenate((x, read), axis=2)
